# revision 32
# baseline (speedup 1.0000x reference)
"""HGCN layer kernel for Trainium2, 8 NeuronCores, row-sharded SPMD.

Reference computation (N=6144, D=512):
    type_sum_a = adj_a @ x ; type_sum_b = adj_b @ x
    attn_a = sigmoid(cat[ts_a, x] @ Wa.T + ba) ; attn_b likewise
    h = x @ W_sa ; s_l = h @ a_sa[:512] ; s_r = h @ a_sa[512:]
    scores[i,j] = s_l[i] + s_r[j]
    e = adj_a * exp(-leaky_relu(scores, 0.01)) ; attn = e / (rowsum(e)+1e-5)
    x_a = attn @ h ; x_b = adj_b @ (x @ W_gcnb) + b_gcnb
    out = sigmoid(attn_a * x_a + attn_b * x_b)

v2.5 strategy (per core, NL=768 local rows):
  - Phase A SHARDED (bf16): each core computes HX = x_local @ R for its
    768 rows, then TWO AllGathers: stats (bf16, tiny, first -> unblocks
    phase C's scalar runahead early) and h (fp8 -> half the collective
    bytes). Rank-block partition-major layout -> 8 batched gather-ins.
  - GCN branch (fp16): t_bT = (adj_b @ x)^T with x j-tiles as PE weights
    and adjacency streaming; epilogue x_b = t_bT.T @ W_gcnb after phase
    C; b_gcnb folded in as a rank-1 matmul; gb rides as appended 1-col
    matmuls. fp8 is numerically unacceptable here (x_b is linear in the
    output), so this branch stays fp16.
  - Attention (fp8 + DoubleRow): e computed transposed [j(part), i]; the
    Scalar engine does Prelu(+s_r bias) and Exp (one act-table set); the
    DVE mask-mult writes e as fp8 into j-tile PAIRS; x_a accumulates
    with DoubleRow matmuls (K=256 per pass, fp8 e pairs x fp8 h pairs).
    rowsum rides as appended 1-col DoubleRow matmuls into an exclusive
    PSUM bank (only the first append starts: start=True clears
    has_written for the WHOLE bank). ga accumulates on the DVE.
"""

import numpy as np
from contextlib import ExitStack

import concourse.bass as bass
import concourse.bacc as bacc
import concourse.mybir as mybir
import concourse.tile as tile

F32 = mybir.dt.float32
BF16 = mybir.dt.bfloat16
FP16 = mybir.dt.float16
FP8 = mybir.dt.float8e4
AF = mybir.ActivationFunctionType
ALU = mybir.AluOpType
PM = mybir.MatmulPerfMode

N_CORES = 8


def build_program(n, d, nl, ba, bb, dt_a=BF16, dt_bc=BF16,
                  dt_g=FP16, use_ag=True, fp8_c=True, lrelu_on_act=True):
    """Build the SPMD Bass program. Returns nc."""
    JT = n // 128   # j tiles (contraction/node axis)
    IT = nl // 128  # local row tiles
    KT = d // 128   # feature k tiles
    NS = 8          # stats cols: 0=s_l 1=s_r 2=zero 3=va 4=wa2x 5=wb2x
    MT = IT
    PT = JT // 2    # j-tile pairs (fp8 DoubleRow)
    dt_h = FP8 if fp8_c else dt_bc

    nc = bacc.Bacc("TRN2", target_bir_lowering=False, debug=False,
                   num_devices=N_CORES)

    xt_dram = nc.dram_tensor("xt", [MT, 128, KT * 128], dt_a, kind="ExternalInput")
    xbf_dram = nc.dram_tensor("xbf", [JT, 128, d], dt_g, kind="ExternalInput")
    r_dram = nc.dram_tensor("rmat", [128, KT * (d + NS)], dt_a, kind="ExternalInput")
    if fp8_c:
        adjat_dram = nc.dram_tensor("adjat", [PT, 128, 2 * nl], dt_bc,
                                    kind="ExternalInput")
    else:
        adjat_dram = nc.dram_tensor("adjat", [JT, 128, nl], dt_bc,
                                    kind="ExternalInput")
    adjbt_dram = nc.dram_tensor("adjbt", [JT, 128, nl], dt_g, kind="ExternalInput")
    wg_dram = nc.dram_tensor("wg", [KT, 128, d + 1], dt_g, kind="ExternalInput")
    brow_dram = nc.dram_tensor("brow", [1, d], dt_g, kind="ExternalInput")
    ident_dram = nc.dram_tensor("ident", [128, 128], F32, kind="ExternalInput")
    out_dram = nc.dram_tensor("out", [nl, d], F32, kind="ExternalOutput")

    def mm(out, lhsT, rhs, start, stop, skip_group_check=False,
           perf_mode=None):
        nc.tensor.matmul(out, lhsT, rhs, start=start, stop=stop,
                         skip_group_check=skip_group_check,
                         perf_mode=perf_mode)

    chn = [(0, 512), (512, nl - 512)] if nl > 512 else [(0, nl)]

    with tile.TileContext(nc) as tc, ExitStack() as ctx:
        const = ctx.enter_context(tc.tile_pool(name="const", bufs=1))

        HP = d + NS
        r_sb = const.tile([128, KT, HP], dt_a, tag="r")
        xbf_sb = const.tile([128, JT * d], dt_g, tag="xbf")
        h_sb = const.tile([128, JT, d], dt_h, tag="h")
        stats_b = const.tile([128, JT * NS], dt_bc, tag="statsb")
        stats_g = const.tile([128, JT * NS], F32, tag="statsg")
        stats_loc = const.tile([128, IT * NS], F32, tag="statsl")
        slb_sb = const.tile([128, nl], F32, tag="slb")
        ga_acc = const.tile([128, nl], F32, tag="ga_acc")
        sl_row = const.tile([1, nl], F32, tag="sl_row")
        tbT_sb = const.tile([128, KT * nl], dt_g, tag="tbT")
        wg_sb = const.tile([128, KT * (d + 1)], dt_g, tag="wg")
        brow_sb = const.tile([1, d], dt_g, tag="brow")
        u_sb = const.tile([128, IT * d], F32, tag="u")
        ident_sb = const.tile([128, 128], F32, tag="ident")
        ones_row = const.tile([1, 128], F32, tag="ones_r")
        ones_16 = const.tile([1, 128], dt_g, tag="ones16")
        ones_colf = const.tile([128, 1], F32, tag="ones_cf")
        onespad_f = const.tile([128, 2], F32, tag="onespad_f")
        ones_e = const.tile([128, 2, 1], dt_h, tag="ones_e")
        neg1 = const.tile([128, 1], F32, tag="neg1")
        ba_sb = const.tile([128, 1], F32, tag="ba")
        bb_sb = const.tile([128, 1], F32, tag="bb")
        gate_sb = const.tile([128, 4 * IT], F32, tag="gate")
        # gate cols: [0:IT]=recip(rowsum) [IT:2IT]=sig_a [2IT:3IT]=sig_b
        # [3IT:4IT]=scratch

        nc.sync.dma_start(out=r_sb.opt(), in_=r_dram[:])
        nc.sync.dma_start(out=ident_sb[:], in_=ident_dram[:])
        nc.vector.memset(ones_row[:], 1.0)
        nc.vector.tensor_copy(ones_16[:], ones_row[:])
        nc.vector.memset(ones_colf[:], 1.0)
        nc.vector.memset(onespad_f[:], 1.0)
        nc.vector.tensor_copy(ones_e.opt(), onespad_f[:])
        nc.vector.memset(neg1[:], -1.0)
        nc.vector.memset(ba_sb[:], float(ba))
        nc.vector.memset(bb_sb[:], float(bb))

        dramp = ctx.enter_context(
            tc.tile_pool(name="dram", bufs=1, space="DRAM"))
        # partition-major rank blocks; stats gathered separately (and
        # first) so phase C's scalar chain unblocks before h arrives.
        st_loc = dramp.tile([128, IT * NS], dt_bc, tag="st_loc",
                            name="st_loc")
        st_full = dramp.tile([N_CORES, 128, IT * NS], dt_bc, tag="st_full",
                             name="st_full", addr_space="Shared")
        h_loc = dramp.tile([128, IT * d], dt_h, tag="h_loc", name="h_loc")
        h_full = dramp.tile([N_CORES, 128, IT * d], dt_h, tag="h_full",
                            name="h_full", addr_space="Shared")

        # ---- Phase A: HX = x @ R for local rows (bf16) ----
        with tc.tile_pool(name="xt_pool", bufs=2) as xtp, \
             tc.tile_pool(name="hx_out", bufs=2) as hxp, \
             tc.tile_pool(name="psA", bufs=2, space="PSUM") as psA:
            for m in range(MT):
                xt_t = xtp.tile([128, KT * 128], dt_a, tag="xt")
                nc.sync.dma_start(out=xt_t[:], in_=xt_dram[m])
                ph = psA.tile([128, d], F32, tag="ph")
                ps = psA.tile([128, NS], F32, tag="ps")
                for k in range(KT):
                    lhsT = xt_t[:, k * 128:(k + 1) * 128]
                    st, sp = (k == 0), (k == KT - 1)
                    mm(ph[:], lhsT, r_sb[:, k, 0:d], st, sp)
                    mm(ps[:], lhsT, r_sb[:, k, d:HP], st, sp)
                nc.vector.tensor_copy(stats_loc[:, m * NS:(m + 1) * NS],
                                      ps[:])
                h_t = hxp.tile([128, d], dt_h, tag="hx")
                st_t = hxp.tile([128, NS], dt_bc, tag="st")
                nc.scalar.copy(h_t[:], ph[:])
                nc.vector.tensor_copy(st_t[:], ps[:])
                nc.gpsimd.dma_start(out=st_loc[:, m * NS:(m + 1) * NS],
                                    in_=st_t[:])
                nc.gpsimd.dma_start(out=h_loc[:, m * d:(m + 1) * d],
                                    in_=h_t[:])

        rg = [list(range(N_CORES))]
        nc.gpsimd.collective_compute("AllGather", mybir.AluOpType.bypass,
                                     replica_groups=rg, ins=[st_loc.opt()],
                                     outs=[st_full.opt()])
        for r in range(N_CORES):
            nc.gpsimd.dma_start(
                out=stats_b[:, r * IT * NS:(r + 1) * IT * NS],
                in_=st_full[r])
        nc.gpsimd.collective_compute("AllGather", mybir.AluOpType.bypass,
                                     replica_groups=rg, ins=[h_loc.opt()],
                                     outs=[h_full.opt()])
        for r in range(N_CORES):
            nc.gpsimd.dma_start(out=h_sb[:, r * IT:(r + 1) * IT, :],
                                in_=h_full[r])

        # ---- Phase A2: build SL broadcast [128, nl] from local s_l ----
        with tc.tile_pool(name="psA2", bufs=1, space="PSUM") as psA2:
            pslc = [psA2.tile([1, c[1]], F32, tag=f"psl{ci}",
                              name=f"psl{ci}")
                    for ci, c in enumerate(chn)]
            for t in range(IT):
                ci, off = divmod(t * 128, 512)
                nc.tensor.matmul(pslc[ci][0:1, off:off + 128],
                                 stats_loc[:, t * NS:t * NS + 1],
                                 ident_sb[:], start=True, stop=True)
            for ci, (o, w) in enumerate(chn):
                nc.vector.tensor_copy(sl_row[0:1, o:o + w], pslc[ci][0:1, :])
            for ci, (o, w) in enumerate(chn):
                pbb = psA2.tile([128, w], F32, tag="pbb")
                nc.tensor.matmul(pbb[:], ones_row[:], sl_row[0:1, o:o + w],
                                 start=True, stop=True)
                nc.vector.tensor_copy(slb_sb[:, o:o + w], pbb[:])

        # progressive f32 stats casts (vector) + ga accumulator init
        for r in range(N_CORES):
            nc.vector.tensor_copy(
                stats_g[:, r * IT * NS:(r + 1) * IT * NS],
                stats_b[:, r * IT * NS:(r + 1) * IT * NS])
        nc.vector.memset(ga_acc[:], 0.0)

        # ---- Phase B: t_bT = (adj_b @ x)^T, x j-tiles as weights ----
        with tc.tile_pool(name="adjB", bufs=6) as adjp, \
             tc.tile_pool(name="psB", bufs=1, space="PSUM") as psB:
            pt_acc = [[psB.tile([128, w], F32, tag=f"pt{dc}_{ci}",
                                name=f"pt{dc}_{ci}")
                       for ci, (o, w) in enumerate(chn)]
                      for dc in range(KT)]
            for j in range(JT):
                nc.sync.dma_start(out=xbf_sb[:, j * d:(j + 1) * d],
                                  in_=xbf_dram[j])
                at = adjp.tile([128, nl], dt_g, tag="adj")
                nc.sync.dma_start(out=at[:], in_=adjbt_dram[j])
                st, sp = (j == 0), (j == JT - 1)
                for dc in range(KT):
                    w_ap = xbf_sb[:, j * d + dc * 128:j * d + (dc + 1) * 128]
                    for ci, (o, w) in enumerate(chn):
                        mm(pt_acc[dc][ci][:], w_ap, at[:, o:o + w], st, sp)
            for dc in range(KT):
                for ci, (o, w) in enumerate(chn):
                    nc.vector.tensor_copy(
                        tbT_sb[:, dc * nl + o:dc * nl + o + w],
                        pt_acc[dc][ci][:])

        for k in range(KT):
            nc.sync.dma_start(out=wg_sb[:, k * (d + 1):(k + 1) * (d + 1)],
                              in_=wg_dram[k])
        nc.sync.dma_start(out=brow_sb[:], in_=brow_dram[:])

        # ---- Phase C + gates-a + u ----
        with tc.tile_pool(name="adjC", bufs=6) as adjp2, \
             tc.tile_pool(name="mC", bufs=10) as mp, \
             tc.tile_pool(name="eC", bufs=4) as ep, \
             tc.tile_pool(name="psC", bufs=1, space="PSUM") as psC:
            pc = [psC.tile([128, d], F32, tag=f"pc{i}", name=f"pc{i}")
                  for i in range(IT)]
            prs = psC.tile([128, 8], F32, tag="prs")
            pgacol = psC.tile([128, 8], F32, tag="pgacol")

            def e_chain(j, at_ap, e_out):
                s_r = stats_g[:, j * NS + 1:j * NS + 2]
                m_t = mp.tile([128, nl], dt_bc if lrelu_on_act else F32,
                              tag="m")
                if lrelu_on_act:
                    nc.scalar.activation(m_t[:], slb_sb[:], AF.Prelu,
                                         bias=s_r, alpha=0.01)
                else:
                    nc.vector.tensor_scalar_add(m_t[:], slb_sb[:], s_r)
                    nc.vector.scalar_tensor_tensor(m_t[:], m_t[:], 0.01,
                                                   m_t[:], op0=ALU.mult,
                                                   op1=ALU.max)
                nc.scalar.activation(m_t[:], m_t[:], AF.Exp, scale=neg1[:])
                nc.vector.tensor_tensor(e_out, m_t[:], at_ap, op=ALU.mult)
                va_f = stats_g[:, j * NS + 3:j * NS + 4]
                nc.vector.scalar_tensor_tensor(ga_acc[:], at_ap, va_f,
                                               ga_acc[:], op0=ALU.mult,
                                               op1=ALU.add)

            if fp8_c:
                for p in range(PT):
                    at2 = adjp2.tile([128, 2, nl], dt_bc, tag="adj")
                    nc.sync.dma_start(out=at2.opt(), in_=adjat_dram[p])
                    e2 = ep.tile([128, 2, nl], FP8, tag="e")
                    for o in range(2):
                        e_chain(2 * p + o, at2[:, o, :], e2[:, o, :])
                    st, sp = (p == 0), (p == PT - 1)
                    for i in range(IT):
                        ew = e2[:, :, i * 128:(i + 1) * 128]
                        mm(pc[i][:], ew, h_sb[:, 2 * p:2 * p + 2, :],
                           st, sp, perf_mode=PM.DoubleRow)
                        mm(prs[:, i:i + 1], ew, ones_e[:],
                           st and i == 0, sp, skip_group_check=True,
                           perf_mode=PM.DoubleRow)
            else:
                for j in range(JT):
                    at = adjp2.tile([128, nl], dt_bc, tag="adj")
                    nc.sync.dma_start(out=at[:], in_=adjat_dram[j])
                    e_t = ep.tile([128, nl], dt_bc, tag="e")
                    e_chain(j, at[:], e_t[:])
                    st, sp = (j == 0), (j == JT - 1)
                    for i in range(IT):
                        ew = e_t[:, i * 128:(i + 1) * 128]
                        mm(pc[i][:], ew, h_sb[:, j, :], st, sp)
                        mm(prs[:, i:i + 1], ew, ones_e[:, 0, :],
                           st and i == 0, sp, skip_group_check=True)

            # ga partition-reduction into per-i gate columns
            for i in range(IT):
                nc.tensor.matmul(pgacol[:, i:i + 1],
                                 ga_acc[:, i * 128:(i + 1) * 128],
                                 ones_colf[:], start=True, stop=True,
                                 skip_group_check=True)
            # gates-a + u = sig_a * recip * x_a_raw
            for i in range(IT):
                nc.vector.tensor_scalar_add(
                    gate_sb[:, 3 * IT + i:3 * IT + i + 1],
                    prs[:, i:i + 1], 1e-5)
                nc.vector.reciprocal(gate_sb[:, i:i + 1],
                                     gate_sb[:, 3 * IT + i:3 * IT + i + 1])
                nc.vector.tensor_tensor(gate_sb[:, 3 * IT + i:3 * IT + i + 1],
                                        pgacol[:, i:i + 1],
                                        stats_loc[:, i * NS + 4:i * NS + 5],
                                        op=ALU.add)
                nc.scalar.activation(gate_sb[:, IT + i:IT + i + 1],
                                     gate_sb[:, 3 * IT + i:3 * IT + i + 1],
                                     AF.Sigmoid, bias=ba_sb[:])
            for i in range(IT):
                nc.vector.tensor_scalar(u_sb[:, i * d:(i + 1) * d], pc[i][:],
                                        gate_sb[:, i:i + 1],
                                        gate_sb[:, IT + i:IT + i + 1],
                                        op0=ALU.mult, op1=ALU.mult)

        # ---- Epilogue: x_b = t_bT.T @ W_gcnb + b ; gb = t_b @ Wb1 ----
        with tc.tile_pool(name="psE", bufs=1, space="PSUM") as psE, \
             tc.tile_pool(name="outD", bufs=2) as outp:
            pxb = [psE.tile([128, d], F32, tag=f"pxb{i}", name=f"pxb{i}")
                   for i in range(IT)]
            pgb = psE.tile([128, 8], F32, tag="pgb")
            for i in range(IT):
                for k in range(KT):
                    lhsT = tbT_sb[:, k * nl + i * 128:k * nl + (i + 1) * 128]
                    mm(pxb[i][:], lhsT, wg_sb[:, k * (d + 1):k * (d + 1) + d],
                       k == 0, False)
                    mm(pgb[:, i:i + 1], lhsT,
                       wg_sb[:, k * (d + 1) + d:(k + 1) * (d + 1)],
                       k == 0, k == KT - 1, skip_group_check=True)
                # fold b_gcnb in as a rank-1 update (ones x brow)
                mm(pxb[i][:], ones_16[:], brow_sb[:], False, True)

            # ---- Phase D2: sig_b gate + combine + output (per-i flow) ----
            for i in range(IT):
                nc.vector.tensor_tensor(gate_sb[:, 3 * IT + i:3 * IT + i + 1],
                                        pgb[:, i:i + 1],
                                        stats_loc[:, i * NS + 5:i * NS + 6],
                                        op=ALU.add)
                nc.scalar.activation(gate_sb[:, 2 * IT + i:2 * IT + i + 1],
                                     gate_sb[:, 3 * IT + i:3 * IT + i + 1],
                                     AF.Sigmoid, bias=bb_sb[:])
                t_t = outp.tile([128, d], F32, tag="t")
                # y = sigmoid(x_b * sig_b + u)
                nc.vector.scalar_tensor_tensor(
                    t_t[:], pxb[i][:], gate_sb[:, 2 * IT + i:2 * IT + i + 1],
                    u_sb[:, i * d:(i + 1) * d], op0=ALU.mult, op1=ALU.add)
                y_t = outp.tile([128, d], F32, tag="y")
                nc.scalar.activation(y_t[:], t_t[:], AF.Sigmoid)
                nc.sync.dma_start(out=out_dram[i * 128:(i + 1) * 128, :],
                                  in_=y_t[:])

    nc.compile()
    return nc


def make_r_matrix(W_sa, a_sa, Wa, Wb, d):
    cols = np.zeros((d, 8), dtype=np.float32)
    cols[:, 0] = W_sa @ a_sa[0, :d]     # s_l weights
    cols[:, 1] = W_sa @ a_sa[0, d:]     # s_r weights
    # col 2 stays zero
    cols[:, 3] = Wa[0, :d]              # va (ga = adj_a @ (x @ Wa1))
    cols[:, 4] = Wa[0, d:]              # wa2x
    cols[:, 5] = Wb[0, d:]              # wb2x
    return np.ascontiguousarray(
        np.concatenate([W_sa, cols], axis=1)).astype(np.float32)


def make_shared_inputs(x, R, W_gcnb, Wb, b_gcnb, n, d, np_a=None,
                       np_g=np.float16):
    import ml_dtypes
    if np_a is None:
        np_a = ml_dtypes.bfloat16
    JT, KT = n // 128, d // 128
    HP = d + 8
    wg = np.concatenate([W_gcnb, Wb[0, :d][:, None]], axis=1)  # [d, d+1]
    rmat = np.ascontiguousarray(
        R.reshape(KT, 128, HP).transpose(1, 0, 2).reshape(128, KT * HP))
    return {
        "xbf": np.ascontiguousarray(x.reshape(JT, 128, d)).astype(np_g),
        "rmat": rmat.astype(np_a),
        "wg": np.ascontiguousarray(wg.reshape(KT, 128, d + 1)).astype(np_g),
        "brow": b_gcnb[None, :].astype(np_g),
        "ident": np.eye(128, dtype=np.float32),
    }


def make_core_inputs(x, adj_a, adj_b, n, d, nl, core, fp8_c=True,
                     np_a=None, np_bc=None, np_g=np.float16):
    import ml_dtypes
    if np_a is None:
        np_a = ml_dtypes.bfloat16
    if np_bc is None:
        np_bc = ml_dtypes.bfloat16
    JT, KT = n // 128, d // 128
    rows = np.arange(core * nl, (core + 1) * nl)
    xl = x[rows]
    MT = nl // 128
    # [m, kk, k*128+mm] = x[rows[m*128+mm], k*128+kk]
    xt = np.ascontiguousarray(
        xl.reshape(MT, 128, KT, 128).transpose(0, 3, 2, 1)
        .reshape(MT, 128, KT * 128))
    adjat = np.ascontiguousarray(adj_a[rows].T).reshape(JT, 128, nl)
    if fp8_c:
        # pair-interleaved: [p, part, o*nl + c] = adjat[2p+o, part, c]
        adjat = np.ascontiguousarray(
            adjat.reshape(JT // 2, 2, 128, nl).transpose(0, 2, 1, 3)
            .reshape(JT // 2, 128, 2 * nl))
    adjbt = np.ascontiguousarray(adj_b[rows].T).reshape(JT, 128, nl)
    return {
        "xt": xt.astype(np_a),
        "adjat": adjat.astype(np_bc),
        "adjbt": adjbt.astype(np_g),
    }


_CACHE = {}


def _install_ntff_hook():
    """Dev-only: register the axon NTFF profile hook so trace=True works."""
    import sys
    import types
    try:
        from antenv import axon_hooks  # noqa: F401
        return
    except ImportError:
        pass
    import antenv
    mod = types.ModuleType("antenv.axon_hooks")
    _h = [None]
    mod.get_axon_ntff_profile_hook = lambda: _h[0]
    mod.set_axon_ntff_profile_hook = lambda hook: _h.__setitem__(0, hook)
    sys.modules["antenv.axon_hooks"] = mod
    antenv.axon_hooks = mod
    from trn_agent_boot.trn_boot import _ntff_profile_via_ctypes
    mod.set_axon_ntff_profile_hook(
        _ntff_profile_via_ctypes("/opt/axon/libaxon_pjrt.so"))


FP8_C = True
LRELU_ON_ACT = True


def kernel(x, adj_a, adj_b, W_sa, a_sa, W_gcnb, b_gcnb, Wa, ba, Wb, bb,
           _trace=False, _trace_kwargs=None):
    from concourse.bass_utils import run_bass_kernel_spmd
    if _trace:
        _install_ntff_hook()

    n, d = x.shape
    nl = n // N_CORES
    R = make_r_matrix(W_sa, a_sa, Wa, Wb, d)

    key = (n, d, nl, float(ba[0]), float(bb[0]), FP8_C, LRELU_ON_ACT, "v25")
    if key not in _CACHE:
        _CACHE[key] = build_program(n, d, nl, float(ba[0]), float(bb[0]),
                                    fp8_c=FP8_C,
                                    lrelu_on_act=LRELU_ON_ACT)
    nc = _CACHE[key]

    shared = make_shared_inputs(x, R, W_gcnb, Wb, b_gcnb, n, d)
    in_maps = []
    for c in range(N_CORES):
        m = dict(shared)
        m.update(make_core_inputs(x, adj_a, adj_b, n, d, nl, c,
                                  fp8_c=FP8_C))
        in_maps.append(m)
    res = run_bass_kernel_spmd(nc, in_maps, list(range(N_CORES)),
                               trace=_trace, **(_trace_kwargs or {}))
    out = np.empty((n, d), dtype=np.float32)
    for c in range(N_CORES):
        out[c * nl:(c + 1) * nl] = res.results[c]["out"]
    if _trace:
        kernel._last_results = res
    return out
